# revision 12
# baseline (speedup 1.0000x reference)
"""CTC loss (projection + log-softmax + alpha recursion) on 8 Trainium2 cores.

Strategy (data-parallel over batch, 2 batches per core, no collectives):
  - big matmul hs@W in bf16 on PE, streaming-sumexp on ACT (accum_out) for the
    log-softmax normalizer
  - small matmul hs@W[:, labels] for the per-state label logits; q ratios
    q = exp(Ls_label - Ls_blank) (blank-probability gauge makes the blank-lane
    update multiplier-free)
  - linear-space alpha recursion on DVE: b' = b + l_m1; l' = (l + b') * q_t,
    with answer-relevant windowed rescale every 16 steps (offset K=e^25)
  - freeze past hlens via q=0 columns; the post-freeze step merges
    b[olen] += l[olen-1], which IS the final logaddexp pair sum
  - per-core output: V_b = log(b[olen]) + sum(log rescales) + sum(blank - lse);
    host: loss = -V, total = sum/16
"""
import math
import sys
import types

import numpy as np
import ml_dtypes

# ---------------------------------------------------------------------------
# axon NTFF-profile shim (lets run_bass_kernel_spmd(trace=True) work) — safe
# no-op if already installed or unavailable.
# ---------------------------------------------------------------------------
def _install_axon_shim():
    if "antenv.axon_hooks" in sys.modules:
        return
    state = {"hook": None}
    mod = types.ModuleType("antenv.axon_hooks")
    mod.set_axon_ntff_profile_hook = lambda h: state.__setitem__("hook", h)
    mod.get_axon_ntff_profile_hook = lambda: state["hook"]
    sys.modules["antenv.axon_hooks"] = mod
    try:
        sys.path.insert(0, "/root/.axon_site/trn_agent_boot")
        try:
            import trn_boot

            hook = trn_boot._ntff_profile_via_ctypes("/opt/axon/libaxon_pjrt.so")
            mod.set_axon_ntff_profile_hook(hook)
        finally:
            sys.path.pop(0)
    except Exception:
        pass


_install_axon_shim()

import concourse.bass as bass
import concourse.bacc as bacc
import concourse.tile as tile
import concourse.mybir as mybir
from concourse.bass_utils import run_bass_kernel_spmd
from concourse.vector_clock import ScopedClock

# ---------------------------------------------------------------------------
# Tile tail-drain patch: this walrus build caps non-EventSemaphore
# instructions at one sync wait, but TileContext._drain_and_barrier funnels
# one wait per pending proc into a single Drain. Re-emit as standalone
# single-wait instructions.
# ---------------------------------------------------------------------------
def _patched_drain_and_barrier(self, tick_clock, wait_clock):
    nc = self.nc
    drain_inst = nc.sync.drain()
    wait_clock.add_sem_waits(
        drain_inst.ins, ScopedClock({None: tick_clock.global_clock})
    )
    waits = list(drain_inst.ins.sync_info.on_wait)
    if len(waits) > 1:
        drain_inst.ins.sync_info = mybir.SyncInfo(on_wait=[], on_update=[])
        by_name = {h.name: h for h in self.sems.allocated().values()}
        for w in waits:
            nc.sync.wait_ge(by_name[w.ant_name], w.wait_value)
        nc.sync.drain()
    nc.all_engine_barrier()
    popped = nc._tile_sem_poison_stack.pop()
    assert popped is self._sem_poison
    nc.clear_and_free_semaphores(list(self.sems.allocated().values()))
    nc.all_engine_barrier()



# ---------------------------------------------------------------------------
B, T, E, V, L = 16, 512, 768, 10000, 100
NCORES = 8
BPC = B // NCORES          # batches per core = 2
S1 = L + 1                 # 101: blank + labels lanes
KT = E // 128              # 6 k-tiles
VC = 500                   # vocab chunk (<=512, 20 chunks)
NVC = V // VC
CAD = 16                   # rescale cadence
K_INV = float(np.exp(-25.0))
NEG = -1e30

F32 = mybir.dt.float32
BF16 = mybir.dt.bfloat16
ADD = mybir.AluOpType.add
SUB = mybir.AluOpType.subtract
MUL = mybir.AluOpType.mult
MAX = mybir.AluOpType.max
AX = mybir.AxisListType.X
EXP = mybir.ActivationFunctionType.Exp
LOG = mybir.ActivationFunctionType.Ln

_CACHE = {}


def _build(tloop, rep_any):
    """Build the SPMD Bass program. tloop = max(hlens); steps t=1..tloop-1."""
    nresc = len([t for t in range(1, tloop) if t % CAD == CAD - 1])
    nc = bacc.Bacc()
    din = {}
    for name, shape, dt in [
        ("hsT", [E, BPC * T], BF16),
        ("w", [E, V], BF16),
        ("wsub", [E, BPC * S1], BF16),
        ("bvec", [1, V], BF16),
        ("bsub", [1, BPC * S1], BF16),
        ("onescol", [1, 128], BF16),
        ("maskcol", [128, 2 * 4], F32),
        ("dmask2", [128, 2 * 4 * BPC], F32),
        ("onehot", [BPC, S1], mybir.dt.uint8),
        ("winmask", [BPC, max(nresc, 1) * S1], mybir.dt.uint8),
        ("repmask", [BPC, L], F32),
    ]:
        din[name] = nc.declare_dram_parameter(name, shape, dt, isOutput=False)
    dout = nc.declare_dram_parameter("out", [BPC, 1], F32, isOutput=True)

    with tile.TileContext(nc) as tc:
        import contextlib

        with contextlib.ExitStack() as ctx:
            cpool = ctx.enter_context(tc.tile_pool(name="consts", bufs=1))
            wpool = ctx.enter_context(tc.tile_pool(name="wstream", bufs=3))
            scrpool = ctx.enter_context(tc.tile_pool(name="scratch", bufs=2))
            qpool = ctx.enter_context(tc.tile_pool(name="qtiles", bufs=2))
            psmm = ctx.enter_context(tc.tile_pool(name="psmm", bufs=6, space="PSUM"))
            psacc = ctx.enter_context(tc.tile_pool(name="psacc", bufs=1, space="PSUM"))
            rpool = ctx.enter_context(tc.tile_pool(name="rec", bufs=4))

            # ---- phase 0: resident loads ----
            hsT = [cpool.tile([128, BPC * T], BF16, tag=f"hsT{k}", name=f"hsT{k}") for k in range(KT)]
            wsub = [cpool.tile([128, BPC * S1], BF16, tag=f"wsub{k}", name=f"wsub{k}") for k in range(KT)]
            hsT_ap = din["hsT"].rearrange("(k p) t -> k p t", p=128)
            wsub_ap = din["wsub"].rearrange("(k p) t -> k p t", p=128)
            for k in range(KT):
                nc.sync.dma_start(hsT[k][:], hsT_ap[k])
                nc.sync.dma_start(wsub[k][:], wsub_ap[k])
            maskcol = cpool.tile([128, 8], F32, tag="maskcol", name="maskcol")
            nc.sync.dma_start(maskcol[:], din["maskcol"][:])
            dmask2 = cpool.tile([128, 16], F32, tag="dmask2", name="dmask2")
            nc.sync.dma_start(dmask2[:], din["dmask2"][:])
            onehot = cpool.tile([BPC, S1], mybir.dt.uint8, tag="onehot", name="onehot")
            nc.sync.dma_start(onehot[:], din["onehot"][:])
            winmask = cpool.tile([BPC, max(nresc, 1) * S1], mybir.dt.uint8, tag="winmask", name="winmask")
            nc.sync.dma_start(winmask[:], din["winmask"][:])
            repmask = cpool.tile([BPC, L], F32, tag="repmask", name="repmask")
            nc.sync.dma_start(repmask[:], din["repmask"][:])
            onescol = cpool.tile([1, 128], BF16, tag="onescol", name="onescol")
            nc.sync.dma_start(onescol[:], din["onescol"][:])
            bsub = cpool.tile([1, BPC * S1], BF16, tag="bsub", name="bsub")
            nc.sync.dma_start(bsub[:], din["bsub"][:])
            bvec_sb = cpool.tile([1, V], BF16, tag="bvec", name="bvec")
            nc.sync.dma_start(bvec_sb[:], din["bvec"][:])

            qf = cpool.tile([BPC, T * S1], BF16, tag="qf", name="qf")
            blank8 = cpool.tile([128, 8], F32, tag="blank8", name="blank8")
            sums_t = cpool.tile([128, 8 * NVC], F32, tag="sums", name="sums")

            # ---- phase 1: small matmul -> q ratios ----
            for bb in range(BPC):
                for rt4 in range(4):
                    rt = bb * 4 + rt4
                    ps = psmm.tile([128, S1], F32, tag="mm", name="mm")
                    # bias row: ps = ones^T (1x128)  x  bsub (1 x S1)
                    nc.tensor.matmul(
                        ps[:], onescol[:, :], bsub[:, bb * S1:(bb + 1) * S1],
                        start=True, stop=False,
                    )
                    for k in range(KT):
                        nc.tensor.matmul(
                            ps[:],
                            hsT[k][:, bb * T + rt4 * 128: bb * T + rt4 * 128 + 128],
                            wsub[k][:, bb * S1:(bb + 1) * S1],
                            start=False, stop=(k == KT - 1),
                        )
                    biascol = scrpool.tile([128, 1], F32, tag="biascol", name="biascol")
                    nc.vector.scalar_tensor_tensor(
                        biascol[:], ps[:, 0:1], -1.0, maskcol[:, rt:rt + 1],
                        op0=MUL, op1=ADD,
                    )
                    nc.scalar.copy(blank8[:, rt:rt + 1], ps[:, 0:1])
                    qt = qpool.tile([128, S1], BF16, tag="qt", name="qt")
                    nc.scalar.activation(qt[:], ps[:], EXP, bias=biascol[:], scale=1.0)
                    nc.sync.dma_start(
                        qf[bb:bb + 1, rt4 * 128 * S1:(rt4 * 128 + 128) * S1], qt[:]
                    )

            # ---- phase 4 core: recursion on DVE (emitted before phase 2's
            # ACT/PE work so the DVE queue is not blocked behind them; Tile
            # schedules by deps, engines run concurrently) ----
            bbuf = [rpool.tile([BPC, S1], BF16, tag=f"bb{i}", name=f"bb{i}") for i in range(3)]
            lbuf = [rpool.tile([BPC, S1 + 1], BF16, tag=f"lb{i}", name=f"lb{i}") for i in range(3)]
            ztile = cpool.tile([BPC, S1], F32, tag="ztile", name="ztile")
            nc.vector.memset(ztile[:], 0.0)
            selt = rpool.tile([BPC, S1], F32, tag="selt", name="selt")
            Ms = cpool.tile([BPC, max(nresc, 1)], F32, tag="Ms")
            nc.vector.memset(Ms[:], 1.0)
            rtile = rpool.tile([BPC, 1], F32, tag="rtile", name="rtile")
            for i in range(3):
                nc.vector.memset(bbuf[i][:], 0.0)
                nc.vector.memset(lbuf[i][:], 0.0)
            nc.vector.tensor_copy(bbuf[0][:, 0:1], qf[:, 0:1])
            nc.vector.tensor_copy(lbuf[0][:, 2:3], qf[:, 1:2])

            cur, nxt, spare = 0, 1, 2
            jresc = 0
            for t in range(1, tloop):
                bc, lc = bbuf[cur], lbuf[cur]
                bn, ln = bbuf[nxt], lbuf[nxt]
                # b' = b + l_m1   (l_m1 for lane k lives at lbuf col k+1)
                nc.vector.tensor_tensor(bn[:], bc[:], lc[:, 1:S1 + 1], ADD)
                if rep_any:
                    trep = rpool.tile([BPC, L], BF16, tag="trep", name="trep")
                    nc.vector.tensor_tensor(trep[:], lc[:, 1:S1], repmask[:], MUL)
                    lt1 = rpool.tile([BPC, L], BF16, tag="lt1", name="lt1")
                    nc.vector.tensor_tensor(lt1[:], lc[:, 2:S1 + 1], bn[:, 0:L], ADD)
                    lt2 = rpool.tile([BPC, L], BF16, tag="lt2", name="lt2")
                    nc.vector.tensor_tensor(lt2[:], lt1[:], trep[:], SUB)
                    nc.vector.tensor_tensor(
                        ln[:, 2:S1 + 1], lt2[:], qf[:, t * S1 + 1:t * S1 + S1], MUL
                    )
                else:
                    lt1 = rpool.tile([BPC, L], BF16, tag="lt1", name="lt1")
                    nc.vector.tensor_tensor(lt1[:], lc[:, 2:S1 + 1], bn[:, 0:L], ADD)
                    nc.vector.tensor_tensor(
                        ln[:, 2:S1 + 1], lt1[:], qf[:, t * S1 + 1:t * S1 + S1], MUL
                    )
                cur, nxt, spare = nxt, spare, cur
                if t % CAD == CAD - 1:
                    bc2, lc2 = bbuf[cur], lbuf[cur]
                    b3, l3 = bbuf[nxt], lbuf[nxt]
                    nc.vector.tensor_copy(selt[:], ztile[:])
                    nc.vector.copy_predicated(
                        selt[:], winmask[:, jresc * S1:(jresc + 1) * S1], bc2[:]
                    )
                    wm = rpool.tile([BPC, 1], F32, tag="wm", name="wm")
                    nc.vector.tensor_reduce(wm[:], selt[:], AX, MAX)
                    nc.vector.tensor_scalar_mul(Ms[:, jresc:jresc + 1], wm[:], K_INV)
                    nc.vector.reciprocal(rtile[:], Ms[:, jresc:jresc + 1])
                    nc.vector.tensor_scalar_mul(b3[:], bc2[:], rtile[:])
                    nc.vector.tensor_scalar_mul(l3[:], lc2[:], rtile[:])
                    cur, nxt, spare = nxt, spare, cur
                    jresc += 1
            # final merge: b[ol] += l[ol-1]  (the logaddexp pair-sum)
            bmerge = rpool.tile([BPC, S1], BF16, tag="bmerge", name="bmerge")
            nc.vector.tensor_tensor(
                bmerge[:], bbuf[cur][:], lbuf[cur][:, 1:S1 + 1], ADD
            )
            nc.vector.tensor_copy(selt[:], ztile[:])
            nc.vector.copy_predicated(selt[:], onehot[:], bmerge[:])
            fv = rpool.tile([BPC, 1], F32, tag="fv", name="fv")
            nc.vector.tensor_reduce(fv[:], selt[:], AX, ADD)

            # ---- phase 2: big matmul + streaming sumexp ----
            w_ap = din["w"].rearrange("(k p) v -> k p v", p=128)
            for vc in range(NVC):
                wt = [wpool.tile([128, VC], BF16, tag=f"wt{k}", name=f"wt{k}") for k in range(KT)]
                for k in range(KT):
                    nc.sync.dma_start(wt[k][:], w_ap[k][:, vc * VC:(vc + 1) * VC])
                for rt in range(8):
                    bb, rt4 = rt // 4, rt % 4
                    ps = psmm.tile([128, VC], F32, tag="mm", name="mm")
                    # bias row via ones x bvec-slice
                    nc.tensor.matmul(
                        ps[:], onescol[:, :], bvec_sb[:, vc * VC:(vc + 1) * VC],
                        start=True, stop=False,
                    )
                    for k in range(KT):
                        nc.tensor.matmul(
                            ps[:],
                            hsT[k][:, bb * T + rt4 * 128: bb * T + rt4 * 128 + 128],
                            wt[k][:],
                            start=False, stop=(k == KT - 1),
                        )
                    scr = scrpool.tile([128, VC], BF16, tag="scr", name="scr")
                    nc.scalar.activation(
                        scr[:], ps[:], EXP, bias=0.0, scale=1.0,
                        accum_out=sums_t[:, rt * NVC + vc: rt * NVC + vc + 1],
                    )

            # ---- phase 3: lse + masked d-sum (post-recursion on DVE) ----
            S8 = cpool.tile([128, 8], F32, tag="S8", name="S8")
            for rt in range(8):
                nc.vector.tensor_reduce(
                    S8[:, rt:rt + 1], sums_t[:, rt * NVC:(rt + 1) * NVC], AX, ADD
                )
            lse8 = cpool.tile([128, 8], F32, tag="lse8", name="lse8")
            nc.scalar.activation(lse8[:], S8[:], LOG, bias=0.0, scale=1.0)
            d8 = cpool.tile([128, 8], F32, tag="d8", name="d8")
            nc.vector.tensor_tensor(d8[:], blank8[:], lse8[:], SUB)
            psd = psacc.tile([BPC, 1], F32, tag="acc", name="acc")
            for rt in range(8):
                nc.tensor.matmul(
                    psd[:], dmask2[:, rt * BPC:(rt + 1) * BPC], d8[:, rt:rt + 1],
                    start=(rt == 0), stop=(rt == 7),
                )

            # ---- tail: V = log(fv) + sum(log Ms) + dsum ----
            flog = rpool.tile([BPC, 1], F32, tag="flog", name="flog")
            nc.scalar.activation(flog[:], fv[:], LOG, bias=0.0, scale=1.0)
            logms = rpool.tile([BPC, max(nresc, 1)], F32, tag="logms")
            nc.scalar.activation(logms[:], Ms[:], LOG, bias=0.0, scale=1.0)
            sm = rpool.tile([BPC, 1], F32, tag="sm", name="sm")
            nc.vector.tensor_reduce(sm[:], logms[:], AX, ADD)
            v1 = rpool.tile([BPC, 1], F32, tag="v1", name="v1")
            nc.vector.tensor_tensor(v1[:], flog[:], sm[:], ADD)
            vout = rpool.tile([BPC, 1], F32, tag="vout", name="vout")
            nc.vector.tensor_tensor(vout[:], v1[:], psd[:], ADD)
            nc.sync.dma_start(dout[:], vout[:])

    nc.compile()
    return nc, nresc


def _prepare(hs_pad, hlens, ys_pad, ys_lens, W, b):
    hs_pad = np.asarray(hs_pad, np.float32)
    hlens = np.asarray(hlens, np.int32)
    ys_pad = np.asarray(ys_pad, np.int32)
    ys_lens = np.asarray(ys_lens, np.int32)
    W = np.asarray(W, np.float32)
    b = np.asarray(b, np.float32)

    tloop = int(hlens.max())
    lab_all = np.where(ys_pad < 0, 0, ys_pad).astype(np.int32)
    rep_any = bool((lab_all[:, 1:] == lab_all[:, :-1]).any())

    key = (tloop, rep_any)
    if key not in _CACHE:
        _CACHE[key] = _build(tloop, rep_any)
    nc, nresc = _CACHE[key]

    bf = ml_dtypes.bfloat16
    w_bf = W.astype(bf)
    bvec = b.reshape(1, V).astype(bf)
    ones = np.ones((1, 128), bf)

    in_maps = []
    for c in range(NCORES):
        gb = [BPC * c + i for i in range(BPC)]
        hs_c = hs_pad[gb]                               # [2, T, E]
        hsT = np.ascontiguousarray(
            hs_c.transpose(2, 0, 1).reshape(E, BPC * T)
        ).astype(bf)
        wsub = np.empty((E, BPC * S1), np.float32)
        bsub = np.empty((1, BPC * S1), np.float32)
        maskcol = np.zeros((128, 8), np.float32)
        dmask2 = np.zeros((128, 16), np.float32)
        onehot = np.zeros((BPC, S1), np.uint8)
        winmask = np.zeros((BPC, max(nresc, 1) * S1), np.uint8)
        repm = np.zeros((BPC, L), np.float32)
        for bb, g in enumerate(gb):
            lab = lab_all[g]
            cols = np.concatenate([[0], lab])
            wsub[:, bb * S1:(bb + 1) * S1] = W[:, cols]
            bsub[0, bb * S1:(bb + 1) * S1] = b[cols]
            Tb = int(hlens[g])
            ol = int(ys_lens[g])
            repm[bb, 1:] = (lab[1:] == lab[:-1]).astype(np.float32)
            onehot[bb, ol] = 1
            for rt4 in range(4):
                t_loc = rt4 * 128 + np.arange(128)
                maskcol[:, bb * 4 + rt4] = np.where(t_loc <= Tb - 1, 0.0, NEG)
                rt = bb * 4 + rt4
                dmask2[:, rt * BPC + bb] = (t_loc <= Tb - 1).astype(np.float32)
            ks = np.arange(S1)
            for j in range(nresc):
                tj = CAD * (j + 1) - 1
                te = min(tj, Tb - 1)
                lo = max(0, ol - (Tb - 1 - te))
                hi = min(100, te + 1)
                winmask[bb, j * S1:(j + 1) * S1] = ((ks >= lo) & (ks <= hi))
        in_maps.append({
            "hsT": hsT,
            "w": w_bf,
            "wsub": wsub.astype(bf),
            "bvec": bvec,
            "bsub": bsub.astype(bf),
            "onescol": ones,
            "maskcol": maskcol,
            "dmask2": dmask2,
            "onehot": onehot,
            "winmask": winmask,
            "repmask": repm,
        })

    return nc, in_maps


def _total(res):
    total = np.float64(0.0)
    for c in range(NCORES):
        v = res.results[c]["out"].reshape(BPC)
        total += np.float64(-v).sum()
    return np.float32(total / B)


def kernel(hs_pad, hlens, ys_pad, ys_lens, W, b):
    nc, in_maps = _prepare(hs_pad, hlens, ys_pad, ys_lens, W, b)
    res = run_bass_kernel_spmd(nc, in_maps, core_ids=list(range(NCORES)))
    return _total(res)


def profile_run(inputs):
    """Run with NTFF tracing; returns (exec_time_ns, loss)."""
    nc, in_maps = _prepare(**inputs)
    res = run_bass_kernel_spmd(
        nc, in_maps, core_ids=list(range(NCORES)), trace=True
    )
    return res.exec_time_ns
